# revision 17
# baseline (speedup 1.0000x reference)
"""Trainium2 Bass kernel for nn_HGNN_link_prediction (sparse slot attention HGNN).

Strategy (8 NeuronCores, nodes sharded, slots replicated):
  - Host: fold layernorm weights into Wk/Wv/Wq, precompute slot-side LN(edges)
    and q = relu(e@Wq+bq) (tiny, [512,256]); transpose x so every big tensor on
    device is feature-major (contraction dim on partitions); x sent as bf16.
  - Device per core (8192 nodes): LN stats via ones-matmuls on PE; c0 = Wc0^T x
    computed up front (fills cold PE, removes x spill); kk/vv/q2 projections;
    slot->node dots slot-major with exp+accum (per-slot exp-sums come free) and
    max8 scans for local per-slot top-10; AllGather of candidates -> global
    threshold; node-major dots recomputed and masked chunk-wise (streaming,
    no big persistent tiles) feeding the updates matmul; bf16 AllReduce of
    updates (+ separate early f32 exp-sum AllReduce); slot MLP replicated;
    stage-2 dots node-major (exp + accum S2 + scans) then slot-major
    (exp + mask + H@edg) streaming; output layers with c0 added via
    identity-matmul; bf16 output.
"""

import os
import sys

sys.path.insert(0, "/opt/trn_rl_repo")

import numpy as np
import ml_dtypes

N_CORES = 8
N, D, NS, HID = 65536, 256, 512, 256
K_N, K_E = 10, 10
EPS = 1e-8
SCALE = D ** -0.5
NL = N // N_CORES          # 8192 local nodes
NM = NL // 128             # 64 node chunks
NT = NL // 512             # 16 free-dim slices of 512
SC = NS // 128             # 4 slot chunks
KC = D // 128              # 2 feature chunks

_CACHE = {}


def _bf(a):
    return np.ascontiguousarray(a.astype(ml_dtypes.bfloat16))


def _f32(a):
    return np.ascontiguousarray(a.astype(np.float32))


def _bias2(b):
    # [256] -> [128, 2] (column m = partitions of dout chunk m)
    return np.ascontiguousarray(b.astype(np.float32).reshape(2, 128).T)


def _ln_np(x, w, b):
    mu = x.mean(-1, keepdims=True)
    var = ((x - mu) ** 2).mean(-1, keepdims=True)
    return (x - mu) / np.sqrt(var + 1e-5) * w + b


def build_module():
    import concourse.bass as bass
    import concourse.mybir as mybir
    from concourse import bacc
    from concourse.tile import TileContext

    dt = mybir.dt
    AF = mybir.ActivationFunctionType
    ALU = mybir.AluOpType

    nc = bacc.Bacc("TRN2", target_bir_lowering=False)

    # ---------------- DRAM I/O ----------------
    xT = nc.dram_tensor("xT", [D, NL], dt.bfloat16, kind="ExternalInput")
    qTs = nc.dram_tensor("qTs", [D, NS], dt.bfloat16, kind="ExternalInput")
    eT = nc.dram_tensor("eT", [D, NS], dt.bfloat16, kind="ExternalInput")
    Wk_p = nc.dram_tensor("Wk_p", [D, D], dt.bfloat16, kind="ExternalInput")
    Wv_p = nc.dram_tensor("Wv_p", [D, D], dt.bfloat16, kind="ExternalInput")
    Wq2_p = nc.dram_tensor("Wq2_p", [D, D], dt.bfloat16, kind="ExternalInput")
    Wk_r = nc.dram_tensor("Wk_r", [D, D], dt.bfloat16, kind="ExternalInput")
    Wm1 = nc.dram_tensor("Wm1", [2 * D, HID], dt.bfloat16, kind="ExternalInput")
    Wm2 = nc.dram_tensor("Wm2", [HID, D], dt.bfloat16, kind="ExternalInput")
    Wconv = nc.dram_tensor("Wconv", [D, D], dt.bfloat16, kind="ExternalInput")
    Wc0 = nc.dram_tensor("Wc0", [D, D], dt.bfloat16, kind="ExternalInput")
    Wc1I = nc.dram_tensor("Wc1I", [D, D], dt.bfloat16, kind="ExternalInput")
    Wt = nc.dram_tensor("Wt", [D, D], dt.bfloat16, kind="ExternalInput")
    bk_p = nc.dram_tensor("bk_p", [128, 2], dt.float32, kind="ExternalInput")
    bq2_p = nc.dram_tensor("bq2_p", [128, 2], dt.float32, kind="ExternalInput")
    bm1 = nc.dram_tensor("bm1", [128, 2], dt.float32, kind="ExternalInput")
    bm2 = nc.dram_tensor("bm2", [128, 2], dt.float32, kind="ExternalInput")
    bk_r = nc.dram_tensor("bk_r", [128, 2], dt.float32, kind="ExternalInput")
    bc01 = nc.dram_tensor("bc01", [128, 2], dt.float32, kind="ExternalInput")
    bt_b = nc.dram_tensor("bt_b", [128, 2], dt.float32, kind="ExternalInput")
    bv_row = nc.dram_tensor("bv_row", [1, D], dt.bfloat16, kind="ExternalInput")
    bconv_row = nc.dram_tensor("bconv_row", [1, D], dt.bfloat16, kind="ExternalInput")
    outT = nc.dram_tensor("outT", [D, NL], dt.bfloat16, kind="ExternalOutput")

    # ---------------- DRAM internal ----------------
    stats_d = nc.dram_tensor("stats_d", [2, NL], dt.float32)
    r_row_d = nc.dram_tensor("r_row_d", [NL], dt.bfloat16)
    mr_row_d = nc.dram_tensor("mr_row_d", [NL], dt.bfloat16)
    ag1_in = nc.dram_tensor("ag1_in", [NS, 16], dt.bfloat16)
    ag1_out = nc.dram_tensor("ag1_out", [N_CORES * NS, 16], dt.bfloat16,
                             addr_space="Shared")
    s_ar_in = nc.dram_tensor("s_ar_in", [1, NS], dt.float32)
    s_ar_out = nc.dram_tensor("s_ar_out", [1, NS], dt.float32, addr_space="Shared")
    tau1_row_d = nc.dram_tensor("tau1_row_d", [NS], dt.bfloat16)
    invs1_row_d = nc.dram_tensor("invs1_row_d", [NS], dt.float32)
    ar_in = nc.dram_tensor("ar_in", [D, NS], dt.bfloat16)
    ar_out = nc.dram_tensor("ar_out", [D, NS], dt.bfloat16, addr_space="Shared")
    tau2_row_d = nc.dram_tensor("tau2_row_d", [NL], dt.bfloat16)
    invs2_row_d = nc.dram_tensor("invs2_row_d", [NL], dt.bfloat16)
    c0_d = nc.dram_tensor("c0_d", [D, NL], dt.bfloat16)

    rg = [list(range(N_CORES))]

    with TileContext(nc) as tc:
        import contextlib
        ctx = contextlib.ExitStack()
        with ctx:
            consts = ctx.enter_context(tc.tile_pool(name="consts", bufs=1))
            bigpool = ctx.enter_context(tc.tile_pool(name="big", bufs=6))
            medpool = ctx.enter_context(tc.tile_pool(name="med", bufs=1))
            scratch = ctx.enter_context(tc.tile_pool(name="scr", bufs=2))
            pspool = ctx.enter_context(tc.tile_pool(name="ps", bufs=4, space="PSUM"))
            pshold = ctx.enter_context(tc.tile_pool(name="psh", bufs=1, space="PSUM"))

            dma = nc.sync.dma_start

            def big(shape, dtype, name):
                return bigpool.tile(shape, dtype, tag="B", name=name, bufs=6)

            # ---- load small constants ----
            def load2(dram, name=None):
                f = dram.shape[1]
                ts = []
                for k in range(dram.shape[0] // 128):
                    t = consts.tile([128, f], dram.dtype, tag=f"c_{name}_{k}",
                                    name=f"c_{name}_{k}")
                    dma(out=t[:, :], in_=dram[k * 128:(k + 1) * 128, :])
                    ts.append(t)
                return ts

            qT_sb = load2(qTs, name="qT")
            eT_sb = load2(eT, name="eT")
            Wk_sb = load2(Wk_p, name="wk")
            Wv_sb = load2(Wv_p, name="wv")
            Wq2_sb = load2(Wq2_p, name="wq2")
            Wkr_sb = load2(Wk_r, name="wkr")
            Wm1_sb = load2(Wm1, name="wm1")
            Wm2_sb = load2(Wm2, name="wm2")
            Wcv_sb = load2(Wconv, name="wcv")
            Wc0_sb = load2(Wc0, name="wc0")
            Wc1_sb = load2(Wc1I, name="wc1")
            Wt_sb = load2(Wt, name="wt")

            def loadb(dram, name):
                t = consts.tile(list(dram.shape), dram.dtype, tag=f"b_{name}",
                                name=f"b_{name}")
                dma(out=t[:, :], in_=dram[:, :])
                return t

            bk_sb = loadb(bk_p, "bk")
            bq2_sb = loadb(bq2_p, "bq2")
            bm1_sb = loadb(bm1, "bm1")
            bm2_sb = loadb(bm2, "bm2")
            bkr_sb = loadb(bk_r, "bkr")
            bc01_sb = loadb(bc01, "bc01")
            bt_sb = loadb(bt_b, "bt")
            bv_sb = loadb(bv_row, "bv")
            bcv_sb = loadb(bconv_row, "bcv")

            ones_k = consts.tile([128, 1], dt.bfloat16, tag="ones_k", name="ones_k")
            nc.vector.memset(ones_k[:, :], 1.0)
            ones_m = consts.tile([1, 128], dt.bfloat16, tag="ones_m", name="ones_m")
            nc.vector.memset(ones_m[:, :], 1.0)
            from concourse.masks import make_identity
            ident = consts.tile([128, 128], dt.bfloat16, tag="ident", name="ident")
            make_identity(nc, ident[:, :])

            # ---- big-tile ring (allocation order defines buffer reuse) ----
            xbf = [big([128, NL], dt.bfloat16, f"xbf{k}") for k in range(KC)]
            xinT = [big([128, NL], dt.bfloat16, f"xin{k}") for k in range(KC)]
            kkT = [big([128, NL], dt.bfloat16, f"kkT{m}") for m in range(KC)]
            # q2T (allocated below) recycles the xbf slots; c0 spills to DRAM.

            # ================= Phase 0: stream x in, stats + c0 =================
            for ns in range(NT):
                sl = slice(ns * 512, (ns + 1) * 512)
                for k in range(KC):
                    dma(out=xbf[k][:, sl], in_=xT[k * 128:(k + 1) * 128, sl])
                ps = pspool.tile([1, 512], dt.float32, tag="ps", name="ps_st")
                ps2 = pspool.tile([1, 512], dt.float32, tag="ps", name="ps_st2")
                for k in range(KC):
                    xsq = scratch.tile([128, 512], dt.bfloat16, tag="xsq", bufs=2,
                                       name="xsq")
                    nc.vector.tensor_tensor(out=xsq[:, :], in0=xbf[k][:, sl],
                                            in1=xbf[k][:, sl], op=ALU.mult)
                    nc.tensor.matmul(ps[:, :], ones_k[:, :], xbf[k][:, sl],
                                     start=(k == 0), stop=(k == KC - 1))
                    nc.tensor.matmul(ps2[:, :], ones_k[:, :], xsq[:, :],
                                     start=(k == 0), stop=(k == KC - 1))
                st = scratch.tile([1, 512], dt.float32, tag="stdr", bufs=2, name="stdr")
                nc.vector.tensor_copy(st[:, :], ps[:, :])
                dma(out=stats_d[0:1, sl], in_=st[:, :])
                st2 = scratch.tile([1, 512], dt.float32, tag="stdr", bufs=2, name="stdr2")
                nc.vector.tensor_copy(st2[:, :], ps2[:, :])
                dma(out=stats_d[1:2, sl], in_=st2[:, :])
                # c0 = Wc0^T x for this slice (independent of LN; keeps PE warm)
                for m in range(KC):
                    msl = slice(m * 128, (m + 1) * 128)
                    psc = pspool.tile([128, 512], dt.float32, tag="ps", name="ps_c0")
                    for k in range(KC):
                        nc.tensor.matmul(psc[:, :], Wc0_sb[k][:, msl], xbf[k][:, sl],
                                         start=(k == 0), stop=(k == KC - 1))
                    c0s = scratch.tile([128, 512], dt.bfloat16, tag="c0s", bufs=3,
                                       name="c0s")
                    nc.scalar.activation(c0s[:, :], psc[:, :], AF.Copy)
                    dma(out=c0_d[msl, sl], in_=c0s[:, :])

            # ---- LN stats -> per-node rstd / mu*rstd ----
            stats_t = medpool.tile([128, 2, NM], dt.float32, tag="stats_t", name="stats_t")
            dma(out=stats_t[:, :, :], in_=stats_d.rearrange("s (p f) -> p s f", p=128))
            mu = medpool.tile([128, NM], dt.float32, tag="mu", name="mu")
            var = medpool.tile([128, NM], dt.float32, tag="var", name="var")
            nc.vector.tensor_scalar(mu[:, :], stats_t[:, 0, :], 1.0 / D, None, ALU.mult)
            nc.vector.tensor_scalar(var[:, :], stats_t[:, 1, :], 1.0 / D, None, ALU.mult)
            musq = medpool.tile([128, NM], dt.float32, tag="musq", name="musq")
            nc.vector.tensor_tensor(out=musq[:, :], in0=mu[:, :], in1=mu[:, :], op=ALU.mult)
            nc.vector.tensor_tensor(out=var[:, :], in0=var[:, :], in1=musq[:, :], op=ALU.subtract)
            nc.vector.tensor_scalar(var[:, :], var[:, :], 1e-5, None, ALU.add)
            rinv = medpool.tile([128, NM], dt.float32, tag="rinv", name="rinv")
            nc.vector.reciprocal(rinv[:, :], var[:, :])
            rstd = medpool.tile([128, NM], dt.float32, tag="rstd", name="rstd")
            nc.scalar.activation(rstd[:, :], rinv[:, :], AF.Sqrt)
            mr = medpool.tile([128, NM], dt.float32, tag="mr", name="mr")
            nc.vector.tensor_tensor(out=mr[:, :], in0=mu[:, :], in1=rstd[:, :], op=ALU.mult)
            rstd_b = medpool.tile([128, NM], dt.bfloat16, tag="rstd_b", name="rstd_b")
            mr_b = medpool.tile([128, NM], dt.bfloat16, tag="mr_b", name="mr_b")
            nc.vector.tensor_copy(rstd_b[:, :], rstd[:, :])
            nc.vector.tensor_copy(mr_b[:, :], mr[:, :])
            dma(out=r_row_d.rearrange("(p f) -> p f", p=128), in_=rstd_b[:, :])
            dma(out=mr_row_d.rearrange("(p f) -> p f", p=128), in_=mr_b[:, :])

            for ns in range(NT):
                sl = slice(ns * 512, (ns + 1) * 512)
                rb = scratch.tile([128, 512], dt.bfloat16, tag="rb", bufs=3, name="rb")
                mrb = scratch.tile([128, 512], dt.bfloat16, tag="mrb", bufs=3,
                                   name="mrb")
                dma(out=rb[:, :], in_=r_row_d[sl].partition_broadcast(128))
                dma(out=mrb[:, :], in_=mr_row_d[sl].partition_broadcast(128))
                for k in range(KC):
                    nc.vector.tensor_tensor(out=xinT[k][:, sl], in0=xbf[k][:, sl],
                                            in1=rb[:, :], op=ALU.mult)
                    nc.vector.tensor_tensor(out=xinT[k][:, sl], in0=xinT[k][:, sl],
                                            in1=mrb[:, :], op=ALU.subtract)

            # ================= Phase 1: kkT projection =================
            for m in range(KC):
                msl = slice(m * 128, (m + 1) * 128)
                for ns in range(NT):
                    sl = slice(ns * 512, (ns + 1) * 512)
                    ps = pspool.tile([128, 512], dt.float32, tag="ps", name="psk")
                    for k in range(KC):
                        nc.tensor.matmul(ps[:, :], Wk_sb[k][:, msl], xinT[k][:, sl],
                                         start=(k == 0), stop=(k == KC - 1))
                    nc.scalar.activation(kkT[m][:, sl], ps[:, :], AF.Relu,
                                         bias=bk_sb[:, m:m + 1])

            # ============ Phase 2a: d1 slot-major scan (local top-k + S) ========
            cands = [medpool.tile([128, 8, NT], dt.bfloat16, tag=f"cand{sc}",
                                  name=f"cand{sc}") for sc in range(SC)]
            S_acc = [medpool.tile([128, NT], dt.float32, tag=f"sacc{sc}",
                                  name=f"sacc{sc}") for sc in range(SC)]
            for sc in range(SC):
                ssl = slice(sc * 128, (sc + 1) * 128)
                for ns in range(NT):
                    sl = slice(ns * 512, (ns + 1) * 512)
                    ps = pspool.tile([128, 512], dt.float32, tag="ps", name="psd1s")
                    for k in range(KC):
                        nc.tensor.matmul(ps[:, :], qT_sb[k][:, ssl], kkT[k][:, sl],
                                         start=(k == 0), stop=(k == KC - 1))
                    w1row = scratch.tile([128, 512], dt.bfloat16, tag="w1row", bufs=2,
                                         name="w1row")
                    nc.scalar.activation(w1row[:, :], ps[:, :], AF.Exp,
                                         accum_out=S_acc[sc][:, ns:ns + 1])
                    nc.vector.max(out=cands[sc][:, :, ns], in_=w1row[:, :])

            # local top-10 candidates per slot
            for sc in range(SC):
                cv = cands[sc][:, :, :]
                t8 = scratch.tile([128, 8], dt.bfloat16, tag="t8", name="t8")
                nc.vector.max(out=t8[:, :], in_=cv)
                zap = scratch.tile([128, 8, NT], dt.bfloat16, tag="zap1", name="zap1")
                nc.vector.match_replace(out=zap[:, :, :], in_to_replace=t8[:, :],
                                        in_values=cv, imm_value=0.0)
                t8b = scratch.tile([128, 8], dt.bfloat16, tag="t8b", name="t8b")
                nc.vector.max(out=t8b[:, :], in_=zap[:, :, :])
                lf = scratch.tile([128, 16], dt.bfloat16, tag=f"loc10_{sc}",
                                  name=f"loc10_{sc}")
                nc.vector.memset(lf[:, :], 0.0)
                nc.vector.tensor_copy(lf[:, 0:8], t8[:, :])
                nc.vector.tensor_copy(lf[:, 8:10], t8b[:, 0:2])
                dma(out=ag1_in[sc * 128:(sc + 1) * 128, :], in_=lf[:, :])

            nc.gpsimd.collective_compute("AllGather", ALU.bypass, replica_groups=rg,
                                         ins=[ag1_in[:, :]], outs=[ag1_out[:, :]])

            # S partial -> DRAM row; S AllReduce queues behind the AllGather but
            # its result is only needed after the updates AllReduce.
            S_p = medpool.tile([128, SC], dt.float32, tag="S_p", name="S_p")
            sdump = scratch.tile([128, NT], dt.float32, tag="sdump", bufs=2,
                                 name="sdump")
            for sc in range(SC):
                nc.scalar.activation(sdump[:, :], S_acc[sc][:, :], AF.Copy,
                                     accum_out=S_p[:, sc:sc + 1])
            dma(out=s_ar_in.rearrange("o (s p) -> p (o s)", p=128), in_=S_p[:, :])
            nc.gpsimd.collective_compute("AllReduce", ALU.add, replica_groups=rg,
                                         ins=[s_ar_in[:, :]], outs=[s_ar_out[:, :]])

            # ---- q2T chunk 0 (independent; fills PE during the AllGather) ----
            q2T = [None, None]
            q2T[0] = big([128, NL], dt.bfloat16, "q2T0")
            for ns in range(NT):
                sl = slice(ns * 512, (ns + 1) * 512)
                ps = pspool.tile([128, 512], dt.float32, tag="ps", name="psq2")
                for k in range(KC):
                    nc.tensor.matmul(ps[:, :], Wq2_sb[k][:, 0:128], xinT[k][:, sl],
                                     start=(k == 0), stop=(k == KC - 1))
                nc.scalar.activation(q2T[0][:, sl], ps[:, :], AF.Identity,
                                     bias=bq2_sb[:, 0:1])

            # ---- merge: global top-10 threshold ----
            tau4 = medpool.tile([128, SC], dt.bfloat16, tag="tau4", name="tau4")
            for sc in range(SC):
                g = scratch.tile([128, N_CORES, 16], dt.bfloat16, tag="gmerge",
                                 name="gmerge")
                dma(out=g[:, :, :],
                    in_=ag1_out.rearrange("(c s p) k -> s p c k", c=N_CORES, p=128)[sc])
                cv = g[:, :, 0:10]
                t8 = scratch.tile([128, 8], dt.bfloat16, tag="gt8", name="gt8")
                nc.vector.max(out=t8[:, :], in_=cv)
                zap = scratch.tile([128, N_CORES, 10], dt.bfloat16, tag="gzap",
                                   name="gzap")
                nc.vector.match_replace(out=zap[:, :, :], in_to_replace=t8[:, :],
                                        in_values=cv, imm_value=0.0)
                t8b = scratch.tile([128, 8], dt.bfloat16, tag="gt8b", name="gt8b")
                nc.vector.max(out=t8b[:, :], in_=zap[:, :, :])
                nc.vector.tensor_copy(tau4[:, sc:sc + 1], t8b[:, 1:2])
            pst = pspool.tile([SC, 128], dt.bfloat16, tag="ps", name="pst1")
            nc.tensor.transpose(pst[:, :], tau4[:, :], ident[:, :])
            taur = scratch.tile([SC, 128], dt.bfloat16, tag="taur", bufs=1, name="taur")
            nc.scalar.activation(taur[:, :], pst[:, :], AF.Copy)
            dma(out=tau1_row_d.rearrange("(f p) -> f p", p=128), in_=taur[:, :])
            tau1_bc = medpool.tile([128, NS], dt.bfloat16, tag="tau1_bc", name="tau1_bc")
            dma(out=tau1_bc[:, :], in_=tau1_row_d[:].partition_broadcast(128))

            # ====== Phase 2b: d1 node-major streaming: vv+exp+mask+psU ==========
            psU = [pshold.tile([128, NS], dt.float32, tag=f"psU{m}", name=f"psU{m}")
                   for m in range(KC)]
            for nm in range(NM):
                nsl = slice(nm * 128, (nm + 1) * 128)
                psv = pspool.tile([128, D], dt.float32, tag="ps", name="psvv")
                for k in range(KC):
                    nc.tensor.matmul(psv[:, :], xinT[k][:, nsl], Wv_sb[k][:, :],
                                     start=(k == 0), stop=False)
                nc.tensor.matmul(psv[:, :], ones_m[:, :], bv_sb[:, :],
                                 start=False, stop=True)
                vvv = scratch.tile([128, D], dt.bfloat16, tag="vvv", bufs=2,
                                   name="vvv")
                nc.scalar.activation(vvv[:, :], psv[:, :], AF.Relu)
                ps = pspool.tile([128, NS], dt.float32, tag="ps", name="psd1t")
                for k in range(KC):
                    nc.tensor.matmul(ps[:, :], kkT[k][:, nsl], qT_sb[k][:, :],
                                     start=(k == 0), stop=(k == KC - 1))
                w1n = scratch.tile([128, NS], dt.bfloat16, tag="w1n", bufs=16,
                                   name="w1n")
                nc.scalar.activation(w1n[:, :], ps[:, :], AF.Exp)
                ge = scratch.tile([128, NS], dt.bfloat16, tag="ge1", bufs=2, name="ge1")
                nc.vector.tensor_tensor(out=ge[:, :], in0=w1n[:, :],
                                        in1=tau1_bc[:, :], op=ALU.is_ge)
                nc.vector.tensor_tensor(out=w1n[:, :], in0=w1n[:, :], in1=ge[:, :],
                                        op=ALU.mult)
                for m in range(KC):
                    nc.tensor.matmul(psU[m][:, :], vvv[:, m * 128:(m + 1) * 128],
                                     w1n[:, :], start=(nm == 0), stop=(nm == NM - 1),
                                     skip_group_check=True)

            # ---- updates partial -> bf16 -> AllReduce ----
            for m in range(KC):
                uf = scratch.tile([128, NS], dt.bfloat16, tag="uf", bufs=1, name="uf")
                nc.vector.tensor_copy(uf[:, :], psU[m][:, :])
                dma(out=ar_in[m * 128:(m + 1) * 128, :], in_=uf[:, :])
            nc.gpsimd.collective_compute("AllReduce", ALU.add, replica_groups=rg,
                                         ins=[ar_in[:, :]], outs=[ar_out[:, :]])

            # ---- q2T chunk 1 (independent of AR; fills PE during it) ----
            q2T[1] = big([128, NL], dt.bfloat16, "q2T1")
            for ns in range(NT):
                sl = slice(ns * 512, (ns + 1) * 512)
                ps = pspool.tile([128, 512], dt.float32, tag="ps", name="psq2")
                for k in range(KC):
                    nc.tensor.matmul(ps[:, :], Wq2_sb[k][:, 128:256], xinT[k][:, sl],
                                     start=(k == 0), stop=(k == KC - 1))
                nc.scalar.activation(q2T[1][:, sl], ps[:, :], AF.Identity,
                                     bias=bq2_sb[:, 1:2])

            # invs1 from the early S AllReduce (ready well before updates AR)
            sres = scratch.tile([128, SC], dt.float32, tag="sres", bufs=1, name="sres")
            dma(out=sres[:, :], in_=s_ar_out.rearrange("o (s p) -> p (o s)", p=128))
            sinv = scratch.tile([128, SC], dt.float32, tag="sinv", bufs=1, name="sinv")
            nc.vector.reciprocal(sinv[:, :], sres[:, :])
            nc.vector.tensor_scalar(sinv[:, :], sinv[:, :],
                                    1.0 / (1.0 + N * EPS), None, ALU.mult)
            dma(out=invs1_row_d.rearrange("(s p) -> p s", p=128), in_=sinv[:, :])
            invs1_bc = medpool.tile([128, NS], dt.float32, tag="invs1_bc",
                                    name="invs1_bc")
            dma(out=invs1_bc[:, :], in_=invs1_row_d[:].partition_broadcast(128))

            updb = [medpool.tile([128, NS], dt.bfloat16, tag=f"updb{m}", name=f"updb{m}")
                    for m in range(KC)]
            for m in range(KC):
                uf2 = scratch.tile([128, NS], dt.bfloat16, tag="uf2", bufs=1, name="uf2")
                dma(out=uf2[:, :], in_=ar_out[m * 128:(m + 1) * 128, :])
                nc.vector.tensor_tensor(out=updb[m][:, :], in0=uf2[:, :],
                                        in1=invs1_bc[:, :], op=ALU.mult)

            # ================= Phase 3: slot side (replicated) =================
            concat_ch = [eT_sb[0], eT_sb[1], updb[0], updb[1]]
            hT = [medpool.tile([128, NS], dt.bfloat16, tag=f"hT{m}", name=f"hT{m}")
                  for m in range(KC)]
            for m in range(KC):
                msl = slice(m * 128, (m + 1) * 128)
                ps = pspool.tile([128, NS], dt.float32, tag="ps", name="psh1")
                for k in range(4):
                    nc.tensor.matmul(ps[:, :], Wm1_sb[k][:, msl], concat_ch[k][:, :],
                                     start=(k == 0), stop=(k == 3))
                nc.scalar.activation(hT[m][:, :], ps[:, :], AF.Relu, bias=bm1_sb[:, m:m + 1])

            e2T = [medpool.tile([128, NS], dt.bfloat16, tag=f"e2T{m}", name=f"e2T{m}")
                   for m in range(KC)]
            for m in range(KC):
                msl = slice(m * 128, (m + 1) * 128)
                ps = pspool.tile([128, NS], dt.float32, tag="ps", name="pse2")
                for k in range(KC):
                    nc.tensor.matmul(ps[:, :], Wm2_sb[k][:, msl], hT[k][:, :],
                                     start=(k == 0), stop=(k == KC - 1))
                nc.scalar.activation(e2T[m][:, :], ps[:, :], AF.Identity,
                                     bias=bm2_sb[:, m:m + 1])

            k2T = [medpool.tile([128, NS], dt.bfloat16, tag=f"k2T{m}", name=f"k2T{m}")
                   for m in range(KC)]
            for m in range(KC):
                msl = slice(m * 128, (m + 1) * 128)
                ps = pspool.tile([128, NS], dt.float32, tag="ps", name="psk2")
                for k in range(KC):
                    nc.tensor.matmul(ps[:, :], Wkr_sb[k][:, msl], e2T[k][:, :],
                                     start=(k == 0), stop=(k == KC - 1))
                nc.scalar.activation(k2T[m][:, :], ps[:, :], AF.Relu, bias=bkr_sb[:, m:m + 1])

            edg = [medpool.tile([128, D], dt.bfloat16, tag=f"edg{sc}", name=f"edg{sc}")
                   for sc in range(SC)]
            for sc in range(SC):
                ssl = slice(sc * 128, (sc + 1) * 128)
                ps = pspool.tile([128, D], dt.float32, tag="ps", name="psedg")
                for k in range(KC):
                    nc.tensor.matmul(ps[:, :], e2T[k][:, ssl], Wcv_sb[k][:, :],
                                     start=(k == 0), stop=False)
                nc.tensor.matmul(ps[:, :], ones_m[:, :], bcv_sb[:, :],
                                 start=False, stop=True)
                nc.scalar.activation(edg[sc][:, :], ps[:, :], AF.Copy)

            # ===== Phase 4a: stage-2 node-major: exp + S2 + scans, per group ====
            # tau2/invS2 row segments are produced per ns-slice group of 4 node
            # chunks so phase 4b can pipeline behind the scans.
            S2 = medpool.tile([128, NM], dt.float32, tag="S2", name="S2")
            stash2 = medpool.tile([128, 8, NM], dt.bfloat16, tag="stash2", name="stash2")
            ti2 = medpool.tile([128, NT, 8], dt.bfloat16, tag="ti2", name="ti2")
            for g in range(NT):
                for j in range(4):
                    nm = 4 * g + j
                    nsl = slice(nm * 128, (nm + 1) * 128)
                    ps = pspool.tile([128, NS], dt.float32, tag="ps", name="psd2n")
                    for k in range(KC):
                        nc.tensor.matmul(ps[:, :], q2T[k][:, nsl], k2T[k][:, :],
                                         start=(k == 0), stop=(k == KC - 1))
                    w2row = scratch.tile([128, NS], dt.bfloat16, tag="w2row", bufs=3,
                                         name="w2row")
                    nc.scalar.activation(w2row[:, :], ps[:, :], AF.Exp,
                                         accum_out=S2[:, nm:nm + 1])
                    t8 = scratch.tile([128, 8], dt.bfloat16, tag="s2t8", name="s2t8")
                    nc.vector.max(out=t8[:, :], in_=w2row[:, :])
                    zap = scratch.tile([128, NS], dt.bfloat16, tag="s2zap", bufs=2,
                                       name="s2zap")
                    nc.vector.match_replace(out=zap[:, :], in_to_replace=t8[:, :],
                                            in_values=w2row[:, :], imm_value=0.0)
                    nc.vector.max(out=stash2[:, :, nm], in_=zap[:, :])
                gsl = slice(4 * g, 4 * (g + 1))
                nc.vector.tensor_copy(ti2[:, g, 0:4], stash2[:, 1, gsl])
                i2f = scratch.tile([128, 4], dt.float32, tag="i2f", bufs=2, name="i2f")
                nc.vector.reciprocal(i2f[:, :], S2[:, gsl])
                nc.vector.tensor_copy(ti2[:, g, 4:8], i2f[:, :])
                pst2 = pspool.tile([8, 128], dt.bfloat16, tag="ps", name="pst2")
                nc.tensor.transpose(pst2[:, :], ti2[:, g, :], ident[:, :])
                t2r = scratch.tile([8, 128], dt.bfloat16, tag="t2r", bufs=2, name="t2r")
                nc.scalar.activation(t2r[:, :], pst2[:, :], AF.Copy)
                dma(out=tau2_row_d[g * 512:(g + 1) * 512].rearrange("(f p) -> f p", p=128),
                    in_=t2r[0:4, :])
                dma(out=invs2_row_d[g * 512:(g + 1) * 512].rearrange("(f p) -> f p", p=128),
                    in_=t2r[4:8, :])

            # ===== Phase 4b/5: stage-2 slot-major + output layers, per slice ====
            for ns in range(NT):
                sl = slice(ns * 512, (ns + 1) * 512)
                tau2s = scratch.tile([128, 512], dt.bfloat16, tag="tau2s", bufs=2,
                                     name="tau2s")
                dma(out=tau2s[:, :], in_=tau2_row_d[sl].partition_broadcast(128))
                invs2s = scratch.tile([128, 512], dt.bfloat16, tag="invs2s", bufs=2,
                                      name="invs2s")
                dma(out=invs2s[:, :], in_=invs2_row_d[sl].partition_broadcast(128))
                w2s = []
                for sc in range(SC):
                    ssl = slice(sc * 128, (sc + 1) * 128)
                    ps = pspool.tile([128, 512], dt.float32, tag="ps", name="psd2s")
                    for k in range(KC):
                        nc.tensor.matmul(ps[:, :], k2T[k][:, ssl], q2T[k][:, sl],
                                         start=(k == 0), stop=(k == KC - 1))
                    wt = scratch.tile([128, 512], dt.bfloat16, tag="w2s", bufs=5,
                                      name="w2s")
                    nc.scalar.activation(wt[:, :], ps[:, :], AF.Exp)
                    ge = scratch.tile([128, 512], dt.bfloat16, tag="ge2", bufs=2,
                                      name="ge2")
                    nc.vector.tensor_tensor(out=ge[:, :], in0=wt[:, :],
                                            in1=tau2s[:, :], op=ALU.is_ge)
                    nc.vector.tensor_tensor(out=wt[:, :], in0=wt[:, :], in1=ge[:, :],
                                            op=ALU.mult)
                    w2s.append(wt)
                nodesT = []
                for m in range(KC):
                    msl = slice(m * 128, (m + 1) * 128)
                    psn = pspool.tile([128, 512], dt.float32, tag="ps", name="psnod")
                    for sc in range(SC):
                        nc.tensor.matmul(psn[:, :], edg[sc][:, msl], w2s[sc][:, :],
                                         start=(sc == 0), stop=(sc == SC - 1))
                    nod = scratch.tile([128, 512], dt.bfloat16, tag="nod", bufs=3,
                                       name="nod")
                    nc.vector.tensor_tensor(out=nod[:, :], in0=psn[:, :],
                                            in1=invs2s[:, :], op=ALU.mult)
                    nodesT.append(nod)
                r1 = []
                for m in range(KC):
                    msl = slice(m * 128, (m + 1) * 128)
                    c0l = scratch.tile([128, 512], dt.bfloat16, tag="c0l", bufs=4,
                                       name="c0l")
                    dma(out=c0l[:, :], in_=c0_d[msl, sl])
                    ps = pspool.tile([128, 512], dt.float32, tag="ps", name="pso1")
                    nc.tensor.matmul(ps[:, :], ident[:, :], c0l[:, :],
                                     start=True, stop=False)
                    for k in range(KC):
                        nc.tensor.matmul(ps[:, :], Wc1_sb[k][:, msl], nodesT[k][:, :],
                                         start=False, stop=(k == KC - 1))
                    rt = scratch.tile([128, 512], dt.bfloat16, tag="r1s", bufs=3,
                                      name="r1s")
                    nc.scalar.activation(rt[:, :], ps[:, :], AF.Relu,
                                         bias=bc01_sb[:, m:m + 1])
                    r1.append(rt)
                for m in range(KC):
                    msl = slice(m * 128, (m + 1) * 128)
                    ps = pspool.tile([128, 512], dt.float32, tag="ps", name="pso2")
                    for k in range(KC):
                        nc.tensor.matmul(ps[:, :], Wt_sb[k][:, msl], r1[k][:, :],
                                         start=(k == 0), stop=(k == KC - 1))
                    ot = scratch.tile([128, 512], dt.bfloat16, tag="ot", bufs=2,
                                      name="ot")
                    nc.scalar.activation(ot[:, :], ps[:, :], AF.Relu,
                                         bias=bt_sb[:, m:m + 1])
                    dma(out=outT[msl, sl], in_=ot[:, :])

    nc.compile()
    return nc


def _prep(inputs):
    """Host-side preparation: fold LN into weights, slot-side precompute."""
    x = _f32(inputs["x"])
    edges = _f32(inputs["edges_mu"]) + np.exp(_f32(inputs["edges_logsigma"])) * _f32(inputs["edges_noise"])
    e = _ln_np(edges, _f32(inputs["ln_e_w"]), _f32(inputs["ln_e_b"]))
    Wq, bq = _f32(inputs["Wq"]), _f32(inputs["bq"])
    q = np.maximum(e @ Wq + bq, 0.0) * SCALE
    lw, lb = _f32(inputs["ln_in_w"]), _f32(inputs["ln_in_b"])
    Wk, bk = _f32(inputs["Wk"]), _f32(inputs["bk"])
    Wv, bv = _f32(inputs["Wv"]), _f32(inputs["bv"])
    Wk_p = lw[:, None] * Wk
    bk_p = lb @ Wk + bk
    Wv_p = lw[:, None] * Wv
    bv_p = lb @ Wv + bv
    Wq2_p = (lw[:, None] * Wq) * SCALE
    bq2_p = (lb @ Wq + bq) * SCALE
    Wc1I = _f32(inputs["Wc1"]) + np.eye(D, dtype=np.float32)
    bc01 = _f32(inputs["bc0"]) + _f32(inputs["bc1"])

    common = {
        "qTs": _bf(q.T), "eT": _bf(e.T),
        "Wk_p": _bf(Wk_p), "Wv_p": _bf(Wv_p), "Wq2_p": _bf(Wq2_p),
        "Wk_r": _bf(Wk), "Wm1": _bf(_f32(inputs["Wm1"])), "Wm2": _bf(_f32(inputs["Wm2"])),
        "Wconv": _bf(_f32(inputs["conv_w"])), "Wc0": _bf(_f32(inputs["Wc0"])),
        "Wc1I": _bf(Wc1I), "Wt": _bf(_f32(inputs["Wt"])),
        "bk_p": _bias2(bk_p), "bq2_p": _bias2(bq2_p),
        "bm1": _bias2(_f32(inputs["bm1"])), "bm2": _bias2(_f32(inputs["bm2"])),
        "bk_r": _bias2(bk), "bc01": _bias2(bc01), "bt_b": _bias2(_f32(inputs["bt"])),
        "bv_row": _bf(bv_p[None, :]), "bconv_row": _bf(_f32(inputs["conv_b"])[None, :]),
    }
    xt = _bf(x.T)  # [256, 65536] bf16
    in_maps = []
    for c in range(N_CORES):
        m = dict(common)
        m["xT"] = np.ascontiguousarray(xt[:, c * NL:(c + 1) * NL])
        in_maps.append(m)
    return in_maps


def _get_nc():
    if "nc" not in _CACHE:
        _CACHE["nc"] = build_module()
    return _CACHE["nc"]


def run(inputs, trace=False, trace_cores=None):
    from concourse.bass_utils import run_bass_kernel_spmd
    nc = _get_nc()
    in_maps = _prep(inputs)
    res = run_bass_kernel_spmd(nc, in_maps, core_ids=list(range(N_CORES)),
                               trace=trace,
                               trace_cores=trace_cores)
    out = np.empty((N, D), np.float32)
    for c in range(N_CORES):
        out[c * NL:(c + 1) * NL, :] = res.results[c]["outT"].T.astype(np.float32)
    return out, res


def kernel(**inputs):
    out, _ = run(inputs)
    return out
